# revision 20
# baseline (speedup 1.0000x reference)
"""Grouped single-step GRU (B=1024, U=8, I=H=512) on 8 trn2 NeuronCores.

Strategy: expert-parallel — core u computes GRU unit u for the whole batch.
v4 (baseline f32r was 77.5us, v3 bf16 63.2us):
  - z/n-gate matmul operands bf16; r-gate runs fp8e4m3 in DoubleRow perf
    mode (2x PE rate, K=256 per instruction). Accuracy: rel_inf ~7e-3 vs
    2e-2 budget (r-gate fp8 measurably free: max error dominated by the
    bf16 z/n paths).
  - weights preloaded on the scalar HWDGE queue in consumption order
    (within a queue DMAs transfer in order - later loads can't starve the
    first slabs); x/h on sync; bias on gpsimd.
  - short f32 junk-matmul burst ramps PE clock/HAM while first DMAs land.
  - per-chunk gate order matches data arrival (bh0) or epilogue drain
    (last chunk: pxn closes last -> only s->n->m->o after final matmul).
  - epilogue o = c*n + z*h with c = sigmoid(-pz-b_z) computed early;
    scalar: 4 ACTs, vector: STT + 3 TT, gpsimd: z*h. All stores on sync.
Output is bf16 outT [H, B] per core; host transposes + upcasts.
"""

import sys

if "/opt/trn_rl_repo" not in sys.path:
    sys.path.insert(0, "/opt/trn_rl_repo")

from contextlib import ExitStack

import numpy as np
import ml_dtypes

import concourse.tile as tile
from concourse import bacc, mybir
from concourse.bass_utils import run_bass_kernel_spmd

B, U, I, H = 1024, 8, 512, 512
NB = 512          # moving-operand width (b-half)
NBH = B // NB     # 2 b-halves
KT = I // 128     # 4 contraction chunks
JT = H // 128     # 4 output-gate partition chunks
N_WARMUP_MM = 7   # f32 junk matmuls (~0.45-0.9us each) to ramp PE clocks
SW = 16.0         # fp8 weight pre-scale (escapes e4m3 subnormals)

F32 = mybir.dt.float32
BF16 = mybir.dt.bfloat16
FP8 = mybir.dt.float8e4
PM = mybir.MatmulPerfMode
AF = mybir.ActivationFunctionType
ALU = mybir.AluOpType
BF = ml_dtypes.bfloat16
E4 = ml_dtypes.float8_e4m3fn

LAST_EXEC_NS = None
TRACE = False
TRACE_DIR = None

_compiled = None


def _ensure_ntff_hook():
    """Provide antenv.axon_hooks + a ctypes NTFF hook when the image lacks
    them, and keep trace artifacts local instead of uploading."""
    import contextlib
    import ctypes
    import types

    from concourse import bass_utils as _bu

    _bu.upload_artifacts = lambda tmpdir: f"local://{tmpdir}"

    try:
        from antenv.axon_hooks import get_axon_ntff_profile_hook  # noqa: F401

        return
    except ImportError:
        pass

    import antenv

    mod = types.ModuleType("antenv.axon_hooks")
    _holder = [None]
    mod.set_axon_ntff_profile_hook = lambda h: _holder.__setitem__(0, h)
    mod.get_axon_ntff_profile_hook = lambda: _holder[0]
    sys.modules["antenv.axon_hooks"] = mod
    antenv.axon_hooks = mod

    lib = ctypes.CDLL("/opt/axon/libaxon_pjrt.so")
    if not hasattr(lib, "axon_start_nrt_profile"):
        return
    lib.axon_start_nrt_profile.argtypes = [
        ctypes.POINTER(ctypes.c_int64),
        ctypes.c_size_t,
    ]
    lib.axon_start_nrt_profile.restype = ctypes.c_int64
    lib.axon_stop_nrt_profile.argtypes = [ctypes.c_char_p]
    lib.axon_stop_nrt_profile.restype = ctypes.c_int64

    @contextlib.contextmanager
    def _hook(output_dir, device_ids):
        import jax

        jax.devices()
        if device_ids:
            ids = (ctypes.c_int64 * len(device_ids))(*device_ids)
            rc = lib.axon_start_nrt_profile(ids, len(device_ids))
        else:
            rc = lib.axon_start_nrt_profile(None, 0)
        if rc != 0:
            raise RuntimeError(f"axon_start_nrt_profile rc={rc}")
        try:
            yield
        finally:
            n = lib.axon_stop_nrt_profile(str(output_dir).encode())
            print(f"ntff profile: {n} file(s) written to {output_dir}")

    mod.set_axon_ntff_profile_hook(_hook)


def _build():
    nc = bacc.Bacc(
        "TRN2",
        target_bir_lowering=False,
        debug=False,
        num_devices=U,
    )
    xT = nc.dram_tensor("xT", [NBH, 128, KT * NB], BF16, kind="ExternalInput").ap()
    hT = nc.dram_tensor("hT", [NBH, 128, KT * NB], BF16, kind="ExternalInput").ap()
    # fp8 copies of x/h for the r-gate DoubleRow matmuls: [bh, p, kk, i, n]
    x8T = nc.dram_tensor("x8T", [NBH, 128, 2, 2, NB], FP8, kind="ExternalInput").ap()
    h8T = nc.dram_tensor("h8T", [NBH, 128, 2, 2, NB], FP8, kind="ExternalInput").ap()
    # bf16 slabs carry z/n gate cols only: [j, p, k*256 + (g-1)*128 + c]
    wih2 = nc.dram_tensor("wih2", [JT, 128, KT * 256], BF16, kind="ExternalInput").ap()
    whh2 = nc.dram_tensor("whh2", [JT, 128, KT * 256], BF16, kind="ExternalInput").ap()
    # fp8 r-gate weights (x16): [j, p, src, kk, i, c]
    wr8 = nc.dram_tensor("wr8", [JT, 128, 2, 2, 2, 128], FP8, kind="ExternalInput").ap()
    biases = nc.dram_tensor("biases", [128, 20], F32, kind="ExternalInput").ap()
    outT = nc.dram_tensor("outT", [H, B], BF16, kind="ExternalOutput").ap()

    with tile.TileContext(nc) as tc, ExitStack() as ctx:
        wpool = ctx.enter_context(tc.tile_pool(name="w", bufs=1))
        xpool = ctx.enter_context(tc.tile_pool(name="x", bufs=1))
        bpool = ctx.enter_context(tc.tile_pool(name="b", bufs=1))
        ppool = ctx.enter_context(tc.tile_pool(name="psum", bufs=2, space="PSUM"))
        epool = ctx.enter_context(tc.tile_pool(name="work", bufs=2))

        # --- warmup operand: gpsimd memset (gpsimd is free earliest) ---
        jnk = bpool.tile([128, 256], F32, tag="jnk")
        nc.gpsimd.memset(jnk[:], 0.0)

        # identity [128,128] bf16 (diag(c - p == 0)) for the last chunk's
        # PE-accumulated t -> pxn add
        it32 = bpool.tile([128, 128], mybir.dt.int32, tag="it32")
        nc.gpsimd.iota(it32[:], pattern=[[1, 128]], base=0, channel_multiplier=-1)
        ident = bpool.tile([128, 128], BF16, tag="ident")
        nc.gpsimd.tensor_scalar(ident[:], it32[:], 0, None, op0=ALU.is_equal)

        # --- load issue ---
        wih_s = {}
        whh_s = {}
        wr8_s = {}
        x_s = {}
        h_s = {}
        x8_s = {}
        h8_s = {}

        def load_wr8(j):
            t = wpool.tile([128, 2, 2, 2, 128], FP8, tag=f"wr8_{j}")
            nc.scalar.dma_start(out=t[:], in_=wr8[j])
            wr8_s[j] = t

        def load_w(j, eng):
            for d, dram, nm in ((wih_s, wih2, "wih"), (whh_s, whh2, "whh")):
                t = wpool.tile([128, KT * 256], BF16, tag=f"{nm}_{j}")
                eng.dma_start(out=t[:], in_=dram[j])
                d[j] = t

        def load_xh(bh, which, eng):
            spec = {
                "x8": (x8_s, x8T, [128, 2, 2, NB], FP8),
                "h8": (h8_s, h8T, [128, 2, 2, NB], FP8),
                "x": (x_s, xT, [128, KT * NB], BF16),
                "h": (h_s, hT, [128, KT * NB], BF16),
            }[which]
            d, dram, shape, dt = spec
            t = xpool.tile(shape, dt, tag=f"{which}_{bh}")
            eng.dma_start(out=t[:], in_=dram[bh])
            d[bh] = t

        # weights on the scalar HWDGE queue in consumption order; x/h on
        # sync; only the tiny bias on gpsimd (its DGE queue is ~3x slower
        # than the scalar/sync HWDGE queues - never put bulk loads there).
        bt = bpool.tile([128, 20], F32, tag="bias")
        load_wr8(0)
        load_xh(0, "x8", nc.sync)
        nc.gpsimd.dma_start(out=bt[:], in_=biases[:])
        load_w(0, nc.scalar)
        load_xh(0, "x", nc.sync)
        load_xh(0, "h", nc.sync)
        load_xh(0, "h8", nc.sync)
        load_wr8(1)
        load_w(1, nc.scalar)
        load_wr8(2)
        load_w(2, nc.scalar)
        load_wr8(3)
        load_w(3, nc.scalar)
        load_xh(1, "x8", nc.sync)
        load_xh(1, "x", nc.sync)
        load_xh(1, "h", nc.sync)
        load_xh(1, "h8", nc.sync)

        # --- PE warmup: f32 junk matmuls (8 x 256-wide ~ 3.5us of PE
        # activity) ramp the clock and trip the HAM boost window right as
        # the first operands land. Shorter bursts delay the boost. ---
        pjnk = ppool.tile([128, 256], F32, tag="pr")
        for _ in range(N_WARMUP_MM):
            nc.tensor.matmul(
                pjnk[:],
                lhsT=jnk[:, 0:128],
                rhs=jnk[:],
                start=True,
                stop=True,
                skip_group_check=True,
            )

        def wsl(ws, j, k, g):
            return ws[j][:, k * 256 + (g - 1) * 128 : k * 256 + (g - 1) * 128 + 128]

        for bh in range(NBH):
            for j in range(JT):
                pr = ppool.tile([128, NB], F32, tag="pr")
                pz = ppool.tile([128, NB], F32, tag="pz")
                pxn = ppool.tile([128, NB], F32, tag="pxn")
                phn = ppool.tile([128, NB], F32, tag="phn")

                def grp(pt, pieces, close=True, open_=True):
                    ops = []
                    for w_d, x_d, g in pieces:
                        ops += [
                            (wsl(w_d, j, k, g), x_d[:, k * NB : (k + 1) * NB])
                            for k in range(KT)
                        ]
                    for i, (w, r) in enumerate(ops):
                        nc.tensor.matmul(
                            pt[:],
                            lhsT=w,
                            rhs=r,
                            start=(open_ and i == 0),
                            stop=(close and i == len(ops) - 1),
                        )

                def grp8(src, ops8, close=True, open_=True):
                    # r-gate fp8 DoubleRow: 2 matmuls of K=256 per operand src
                    for kk in range(2):
                        nc.tensor.matmul(
                            pr[:],
                            lhsT=wr8_s[j][:, src, kk],
                            rhs=ops8[:, kk],
                            start=(open_ and kk == 0),
                            stop=(close and kk == 1),
                            perf_mode=PM.DoubleRow,
                        )

                xx, hh = x_s[bh], h_s[bh]
                if bh == 0:
                    # first pass: x-parts first (x/wih land before h/whh);
                    # h8 arrives last so pr closes last (off drain path).
                    grp8(0, x8_s[bh], close=False)
                    grp(pz, [(wih_s, xx, 1)], close=False)
                    grp(pxn, [(wih_s, xx, 2)])
                    grp(pz, [(whh_s, hh, 1)], open_=False)
                    grp(phn, [(whh_s, hh, 2)])
                    grp8(1, h8_s[bh], open_=False)
                elif not (bh == NBH - 1 and j == JT - 1):
                    # weights cached; pxn last -> shortest post-matmul drain
                    grp8(0, x8_s[bh], close=False)
                    grp8(1, h8_s[bh], open_=False)
                    grp(pz, [(wih_s, xx, 1), (whh_s, hh, 1)])
                    grp(phn, [(whh_s, hh, 2)])
                    grp(pxn, [(wih_s, xx, 2)])
                else:
                    # last chunk: phn early so t=(phn+b)*r is ready before
                    # pxn closes; t is then PE-accumulated into pxn via the
                    # identity matmul, leaving only n->m->o after the stream.
                    grp8(0, x8_s[bh], close=False)
                    grp8(1, h8_s[bh], open_=False)
                    grp(phn, [(whh_s, hh, 2)])
                    grp(pz, [(wih_s, xx, 1), (whh_s, hh, 1)])
                    grp(pxn, [(wih_s, xx, 2)], close=False)

                r_t = epool.tile([128, NB], BF16, tag="r")
                z_t = epool.tile([128, NB], BF16, tag="z")
                c_t = epool.tile([128, NB], BF16, tag="c")
                zh_t = epool.tile([128, NB], BF16, tag="zh")
                t_t = epool.tile([128, NB], BF16, tag="t")
                s_t = epool.tile([128, NB], F32, tag="s")
                n_t = epool.tile([128, NB], BF16, tag="n")
                m_t = epool.tile([128, NB], BF16, tag="m")
                o_t = epool.tile([128, NB], BF16, tag="o")
                h_j = hh[:, j * NB : (j + 1) * NB]

                last = bh == NBH - 1 and j == JT - 1

                # early (off drain chain): r, z, c=1-z, zh=z*h, t=(phn+b)*r
                nc.scalar.activation(
                    r_t[:], pr[:], AF.Sigmoid,
                    bias=bt[:, j : j + 1], scale=1.0 / SW,
                )
                nc.scalar.activation(
                    z_t[:], pz[:], AF.Sigmoid, bias=bt[:, 4 + j : 5 + j]
                )
                nc.scalar.activation(
                    c_t[:], pz[:], AF.Sigmoid,
                    bias=bt[:, 16 + j : 17 + j], scale=-1.0,
                )
                # zh off the gpsimd engine for the last chunk: gpsimd TT is
                # ~2x slower and would sit right on the drain chain
                zh_eng = nc.vector if last else nc.gpsimd
                zh_eng.tensor_tensor(zh_t[:], z_t[:], h_j[:], op=ALU.mult)
                nc.vector.scalar_tensor_tensor(
                    t_t[:],
                    phn[:],
                    bt[:, 12 + j : 13 + j],
                    r_t[:],
                    op0=ALU.add,
                    op1=ALU.mult,
                )
                if last:
                    # s = pxn + t folded into PSUM by an identity matmul so
                    # the post-stream chain is just n -> m -> o -> store
                    nc.tensor.matmul(
                        pxn[:], lhsT=ident[:], rhs=t_t[:], start=False, stop=True
                    )

                # drain chain after pxn closes. Last chunk: [384, 128]
                # strips; the small final store rides the scalar queue so
                # both stores issue in parallel.
                bounds = [(0, 384, nc.sync), (384, 512, nc.scalar)] if last else [
                    (0, NB, nc.sync)
                ]
                for c0, c1, eng in bounds:
                    cs = slice(c0, c1)
                    if last:
                        sv = pxn
                    else:
                        nc.vector.tensor_tensor(
                            s_t[:, cs], t_t[:, cs], pxn[:, cs], op=ALU.add
                        )
                        sv = s_t
                    nc.scalar.activation(
                        n_t[:, cs], sv[:, cs], AF.Tanh,
                        bias=bt[:, 8 + j : 9 + j],
                    )
                    nc.vector.tensor_tensor(
                        m_t[:, cs], c_t[:, cs], n_t[:, cs], op=ALU.mult
                    )
                    nc.vector.tensor_tensor(
                        o_t[:, cs], m_t[:, cs], zh_t[:, cs], op=ALU.add
                    )
                    eng.dma_start(
                        out=outT[
                            j * 128 : (j + 1) * 128,
                            bh * NB + c0 : bh * NB + c1,
                        ],
                        in_=o_t[:, cs],
                    )

    nc.compile()
    return nc


def _get_nc():
    global _compiled
    if _compiled is None:
        _compiled = _build()
    return _compiled


def _prep_in_maps(inputs, hidden, W_ih, W_hh, b_ih, b_hh):
    def pack_xh(a):
        # [B, U, I] -> [U, bh, p, k*NB + b]: tile[p, k*NB+b] = a[bh*NB+b, u, k*128+p]
        a = np.asarray(a, dtype=np.float32)
        a5 = a.reshape(NBH, NB, U, KT, 128)  # [bh, b, u, k, p]
        t = np.ascontiguousarray(a5.transpose(2, 0, 4, 3, 1))  # [u, bh, p, k, b]
        return t.reshape(U, NBH, 128, KT * NB).astype(BF), t.reshape(
            U, NBH, 128, 2, 2, NB
        ).astype(E4)

    x, x8 = pack_xh(inputs)
    h, h8 = pack_xh(hidden)

    def reorder_w(W):
        # z/n cols only -> per-unit [J, 128, K*256]: slab[j, p, k*256+(g-1)*128+c]
        wT = np.asarray(W, dtype=np.float32).transpose(0, 2, 1)  # [U, I, G]
        w5 = wT.reshape(U, KT, 128, 3, JT, 128)[:, :, :, 1:]  # [u, k, p, g', j, c]
        return (
            np.ascontiguousarray(w5.transpose(0, 4, 2, 1, 3, 5))
            .reshape(U, JT, 128, KT * 256)
            .astype(BF)
        )

    def reorder_wr8(Wi, Wh):
        # r-gate cols, fp8 x SW: [u, j, p, src, kk, i, c]
        out = np.empty((U, JT, 128, 2, 2, 2, 128), dtype=E4)
        for src, W in enumerate((Wi, Wh)):
            wT = np.asarray(W, dtype=np.float32).transpose(0, 2, 1)[:, :, :H]
            w6 = (wT * SW).reshape(U, 2, 2, 128, JT, 128)  # [u, kk, i, p, j, c]
            out[:, :, :, src] = w6.transpose(0, 4, 3, 1, 2, 5).astype(E4)
        return out

    wih = reorder_w(W_ih)
    whh = reorder_w(W_hh)
    wr8 = reorder_wr8(W_ih, W_hh)
    bi = np.asarray(b_ih, dtype=np.float32)
    bh = np.asarray(b_hh, dtype=np.float32)
    brz = bi[:, : 2 * H] + bh[:, : 2 * H]  # r and z biases combine
    b_in = bi[:, 2 * H :]
    b_hn = bh[:, 2 * H :]
    in_maps = []
    for u in range(U):
        # [128, 20] tile: column cls*4 + j holds bias_cls[j*128 + p]
        # classes: b_r, b_z, b_in, b_hn, -b_z (for c = sigmoid(-pz - b_z))
        bb = np.stack(
            [brz[u, :H], brz[u, H:], b_in[u], b_hn[u], -brz[u, H:]], axis=0
        )
        bb = bb.reshape(5, 4, 128).transpose(2, 0, 1).reshape(128, 20)
        in_maps.append(
            {
                "xT": x[u],
                "hT": h[u],
                "x8T": x8[u],
                "h8T": h8[u],
                "wih2": wih[u],
                "whh2": whh[u],
                "wr8": wr8[u],
                "biases": np.ascontiguousarray(bb),
            }
        )
    return in_maps


def kernel(inputs, hidden, W_ih, W_hh, b_ih, b_hh):
    global LAST_EXEC_NS
    nc = _get_nc()
    in_maps = _prep_in_maps(inputs, hidden, W_ih, W_hh, b_ih, b_hh)
    kwargs = {}
    if TRACE:
        _ensure_ntff_hook()
        if TRACE_DIR is not None:
            import os

            os.makedirs(TRACE_DIR, exist_ok=True)
            kwargs["tmpdir"] = TRACE_DIR
    res = run_bass_kernel_spmd(nc, in_maps, list(range(U)), trace=TRACE, **kwargs)
    LAST_EXEC_NS = res.exec_time_ns
    out = np.empty((B, U, H), dtype=np.float32)
    for u in range(U):
        out[:, u, :] = res.results[u]["outT"].astype(np.float32).T
    return out
